# revision 21
# baseline (speedup 1.0000x reference)
"""Trainium2 Bass kernel for nn_AttentionLayer (B=128,H=16,L=64,E=128, C=2048).

out[b,l,:] = (softmax(0.1 * q_bh @ k_bh^T) @ v_bh  for h) . W^T + bias

Strategy: pure data-parallel over batch across 8 NeuronCores (16 batches
per core, no collectives).  Per core, in 8 two-batch blocks:
  - q/k/v DMA'd token-major (f32, sync queue), cast to fp16 on DVE.
    fp16 (not bf16): same PE speed, ~8x lower quantization error.
  - attention per (batch, head-pair) group in "scores^T" orientation:
    per-block q/k PE-transposes into one [e, hl] buffer, then one full
    128x128 k^T q matmul per group (diagonal 64x64 blocks are the two
    heads; off-diagonal garbage is never read),
  - softmax without max-subtraction (|0.1*s| < ~8 so exp cannot
    overflow): exp writes the diagonal blocks of a pre-zeroed ring slot
    so U = exp @ [v|1] can contract all 128 partitions in one matmul;
    the ones-column yields the rowsum; normalization is a per-partition
    tensor_scalar multiply in token-major form; V^T via a 16-bit PE
    transpose (the old f32 transpose was 2x the PE cycles),
  - output projection  out = V @ W^T + b  as K=2048 accumulated matmuls
    emitted n-slice-major (psum [128 tok, 512 n]); proj matmuls
    interleave between the NEXT blocks' attention groups (deque of
    generators) to keep the PE dense,
  - W^T prep as an engine-isolated pipeline that chases the W stream:
    W chunks on the gpsimd SWDGE queue (f32, concurrent with the sync
    queue's q/k/v loads), f32->fp16 cast on the Pool engine, PE
    transposes + psum->sbuf copies emitted as generators interleaved
    into blocks 0-2's attention groups,
  - bias add on DVE (Pool cannot read PSUM), store on the gpsimd queue.
"""

from collections import deque

import numpy as np

import concourse.bass as bass
import concourse.mybir as mybir
import concourse.tile as tile
from concourse import bacc
from concourse.bass_utils import run_bass_kernel_spmd
from concourse.masks import make_identity

N_CORES = 8
B, H, L, E = 128, 16, 64, 128
C = H * E                 # 2048
BPC = B // N_CORES        # 16 batches per core
NBLK = BPC // 2           # 8 two-batch blocks per core
G = H // 2                # 8 head-pair groups per batch
SCALE = 0.1
F32 = mybir.dt.float32
DT16 = mybir.dt.float16

# kept for test.py compatibility
ATT_MODE = "fp16"
MM3_MODE = "fp16"

EXPR = 8                  # exp ring slots


def emit(ctx, nc, tc, q_d, k_d, v_d, w_d, b_d, o_d):
    # DRAM views: [p, b, g, e] where token row (h*64+l) = g*128 + p
    qv = q_d.rearrange("b h l e -> b (h l) e").rearrange("b (g p) e -> p b g e", p=128)
    kv = k_d.rearrange("b h l e -> b (h l) e").rearrange("b (g p) e -> p b g e", p=128)
    vv = v_d.rearrange("b h l e -> b (h l) e").rearrange("b (g p) e -> p b g e", p=128)

    const = ctx.enter_context(tc.tile_pool(name="const", bufs=1))
    wst = ctx.enter_context(tc.tile_pool(name="wst", bufs=3))
    qkvf = ctx.enter_context(tc.tile_pool(name="qkvf", bufs=5))
    qkvb = ctx.enter_context(tc.tile_pool(name="qkvb", bufs=2))
    qktp = ctx.enter_context(tc.tile_pool(name="qktp", bufs=2))
    vtp = ctx.enter_context(tc.tile_pool(name="vtp", bufs=2))
    asml = ctx.enter_context(tc.tile_pool(name="asml", bufs=6))
    outp = ctx.enter_context(tc.tile_pool(name="outp", bufs=3))

    pat = ctx.enter_context(tc.tile_pool(name="pat", bufs=6, space="PSUM"))
    pps = pat
    pmm3 = ctx.enter_context(tc.tile_pool(name="pmm3", bufs=2, space="PSUM"))

    # ---- constants ----
    identity16 = const.tile([128, 128], DT16, tag="id16")
    make_identity(nc, identity16)

    # ring of pre-zeroed exp tiles: only the two diagonal 64x64 blocks are
    # ever (re)written, so the off-diagonal blocks stay zero and MM2 can
    # contract over the full 128 partitions without mixing the two heads
    exp_ring = const.tile([128, EXPR, 128], DT16, tag="expr")
    nc.vector.memset(exp_ring, 0.0)

    bias_bc = const.tile([128, C], F32, tag="bias")
    b_bcast = bass.AP(tensor=b_d.tensor, offset=b_d.offset, ap=[[0, 128]] + list(b_d.ap))
    nc.gpsimd.dma_start(out=bias_bc, in_=b_bcast)

    # W^T in fp16: wt_sb[p, kk, n] = W[n, kk*128+p]
    wt_sb = const.tile([128, H, C], DT16, tag="wt")

    # ---- block loads: DMA f32 (sync queue), cast to fp16 (DVE), PE
    # transposes of q/k into one [e, hl] buffer ----
    def load_block(m):
        qf = qkvf.tile([128, 2, G, 128], F32, tag="qkvf")
        kf = qkvf.tile([128, 2, G, 128], F32, tag="qkvf")
        vf = qkvf.tile([128, 2, G, 128], F32, tag="qkvf")
        nc.sync.dma_start(out=qf, in_=qv[:, 2 * m : 2 * m + 2, :, :])
        nc.sync.dma_start(out=kf, in_=kv[:, 2 * m : 2 * m + 2, :, :])
        nc.sync.dma_start(out=vf, in_=vv[:, 2 * m : 2 * m + 2, :, :])
        qb = qkvb.tile([128, 2, G, 128], DT16, tag="qb")
        kb = qkvb.tile([128, 2, G, 128], DT16, tag="kb")
        vb = qkvb.tile([128, 2, G, 132], DT16, tag="vb")
        for bb in range(2):
            nc.vector.tensor_copy(qb[:, bb, :, :], qf[:, bb, :, :])
            nc.vector.tensor_copy(kb[:, bb, :, :], kf[:, bb, :, :])
            nc.vector.tensor_copy(vb[:, bb, :, 0:128], vf[:, bb, :, :])
        nc.vector.memset(vb[:, :, :, 128:129], 1.0)
        return qb, kb, vb

    # ---- W prep: engine-isolated stream.  gpsimd queue: W f32 loads.
    # Pool: f32->fp16 cast.  PE: transposes (as generators interleaved into
    # blocks 0-2's attention).  psum->sbuf copies alternate scalar/DVE. ----
    def wprep_load(nt):
        wn_f = wst.tile([128, C], F32, tag="wnf", name=f"wnf{nt}")
        nc.gpsimd.dma_start(out=wn_f, in_=w_d[nt * 128 : (nt + 1) * 128, :])
        wn_c = wst.tile([128, C], DT16, tag="wnc", name=f"wnc{nt}")
        for hh in range(2):
            nc.gpsimd.tensor_copy(
                wn_c[:, hh * 1024 : (hh + 1) * 1024],
                wn_f[:, hh * 1024 : (hh + 1) * 1024],
            )
        return wn_c

    def wprep_emitter(nt, wn_c):
        for kk in range(0, H, 2):
            tp = pps.tile([128, 256], DT16, tag="at", name="wtp")
            nc.tensor.transpose(
                tp[:, 0:128], wn_c[:, kk * 128 : (kk + 1) * 128], identity16
            )
            yield
            nc.tensor.transpose(
                tp[:, 128:256], wn_c[:, (kk + 1) * 128 : (kk + 2) * 128], identity16
            )
            yield
            if (kk // 2) % 2 == 0:
                nc.scalar.copy(
                    wt_sb[:, kk : kk + 2, nt * 128 : (nt + 1) * 128],
                    tp.rearrange("p (a b) -> p a b", a=2),
                )
            else:
                nc.vector.tensor_copy(
                    wt_sb[:, kk : kk + 2, nt * 128 : (nt + 1) * 128],
                    tp.rearrange("p (a b) -> p a b", a=2),
                )
            yield
        # chain: queue the next W chunk's load + generator (ring depth 3)
        if nt + 3 < H:
            wpend.append(wprep_emitter(nt + 3, wprep_load(nt + 3)))

    # ---- output projection, n-slice-major; generator interleaves with the
    # NEXT blocks' attention groups ----
    def proj_emitter(m, vt):
        for nn in range(4):
            ps = pmm3.tile([128, 512], F32, tag="mm3", name=f"ps{nn}")
            for kk in range(H):
                nc.tensor.matmul(
                    ps,
                    vt[:, kk, :],
                    wt_sb[:, kk, nn * 512 : (nn + 1) * 512],
                    start=(kk == 0), stop=(kk == 15),
                )
                yield
            oth = outp.tile([128, 512], F32, tag="ot", name="oth")
            nc.vector.tensor_add(oth, ps, bias_bc[:, nn * 512 : (nn + 1) * 512])
            nc.gpsimd.dma_start(
                out=o_d[m * 128 : (m + 1) * 128, nn * 512 : (nn + 1) * 512],
                in_=oth,
            )
            yield

    def drain_steps(dq, k):
        while k > 0 and dq:
            try:
                next(dq[0])
                k -= 1
            except StopIteration:
                dq.popleft()

    def drain_all(dq):
        while dq:
            drain_steps(dq, 1 << 30)

    pending = deque()   # proj generators
    wpend = deque()     # wprep generators

    with nc.named_scope("load0"):
        blk_tiles = load_block(0)
    for nt in range(3):
        wpend.append(wprep_emitter(nt, wprep_load(nt)))

    # blocks 0-1 interleave wprep generator steps into attention groups;
    # each nt generator also queues the nt+3 W load when it finishes.
    # All wprep must be EMITTED before proj(0)'s later slices drain (Tile
    # only sees writers emitted before a reader), hence drain_all below.
    WSTEPS = {0: 12, 1: 16}

    # ---- per-block pipeline ----
    for m in range(NBLK):
        qb, kb, vb = blk_tiles
        vt = vtp.tile([128, H, 128], DT16, tag="vt")
        with nc.named_scope(f"attn{m}"):
            # batch-transpose this block's q and k up front
            qkt = qktp.tile([128, 2, G, 256], DT16, tag="qkt")
            for bb in range(2):
                for g in range(G):
                    trp = pps.tile([128, 256], DT16, tag="at", name="trp")
                    nc.tensor.transpose(trp[:, 0:128], qb[:, bb, g, :], identity16)
                    nc.tensor.transpose(trp[:, 128:256], kb[:, bb, g, :], identity16)
                    nc.vector.tensor_copy(qkt[:, bb, g, :], trp)
            for bb in range(2):
                for g in range(G):
                    drain_steps(wpend, WSTEPS.get(m, 0))
                    drain_steps(pending, 2 if m <= 1 else 5)
                    qT2 = qkt[:, bb, g, 0:128]
                    kT2 = qkt[:, bb, g, 128:256]

                    # One psum bank holds this group's scores^T (cols 0:128),
                    # U' = exp@[v|1] (cols 128:257), V^T fp16 (bitcast region)
                    at = pat.tile([128, 392], F32, tag="at")
                    scT = at[:, 0:128]
                    nc.tensor.matmul(scT, kT2, qT2, start=True, stop=True)

                    # exp(scale * scores^T) into a pre-zeroed ring slot
                    expT = exp_ring[:, (bb * G + g) % EXPR, :]
                    for lo, hi in ((0, 64), (64, 128)):
                        nc.scalar.activation(
                            expT[lo:hi, lo:hi], scT[lo:hi, lo:hi],
                            mybir.ActivationFunctionType.Exp, scale=SCALE,
                        )

                    # U = exp @ [v | 1]  -> token-major U plus rowsum column
                    U2p = at[:, 128:257]
                    nc.tensor.matmul(
                        U2p, expT, vb[:, bb, g, 0:129], start=True, stop=True
                    )

                    # normalize token-major (per-partition scalar), fp16 out
                    r2 = asml.tile([128, 1], F32, tag="r2")
                    nc.vector.reciprocal(r2, U2p[:, 128:129])
                    V2 = asml.tile([128, 128], DT16, tag="V2")
                    nc.vector.tensor_scalar_mul(V2, U2p[:, 0:128], r2)

                    # 16-bit PE transpose of V into c-major layout
                    VT2p = at[:, 260:324].bitcast(DT16)
                    nc.tensor.transpose(VT2p, V2, identity16)
                    tok = bb * 64
                    nc.vector.tensor_copy(
                        vt[:, 2 * g : 2 * g + 2, tok : tok + 64],
                        VT2p.rearrange("p (a b) -> p a b", a=2),
                    )

        # drain any wprep leftovers before proj(0)'s later slices emit
        if m == 1:
            drain_all(wpend)
        # prefetch next block
        if m + 1 < NBLK:
            with nc.named_scope(f"load{m + 1}"):
                blk_tiles = load_block(m + 1)
        pending.append(proj_emitter(m, vt))
        if m == NBLK - 1:
            drain_all(pending)


def build(att_mode=ATT_MODE, mm3_mode=MM3_MODE):
    import contextlib

    nc = bacc.Bacc("TRN2", target_bir_lowering=False, debug=False)
    q_d = nc.dram_tensor("queries", [BPC, H, L, E], F32, kind="ExternalInput").ap()
    k_d = nc.dram_tensor("keys", [BPC, H, L, E], F32, kind="ExternalInput").ap()
    v_d = nc.dram_tensor("values", [BPC, H, L, E], F32, kind="ExternalInput").ap()
    w_d = nc.dram_tensor("W", [C, C], F32, kind="ExternalInput").ap()
    b_d = nc.dram_tensor("b", [C], F32, kind="ExternalInput").ap()
    o_d = nc.dram_tensor("out", [BPC * L, C], F32, kind="ExternalOutput").ap()

    with tile.TileContext(nc) as tc:
        with contextlib.ExitStack() as ctx:
            emit(ctx, nc, tc, q_d, k_d, v_d, w_d, b_d, o_d)
    nc.compile()
    return nc


_NC_CACHE = {}


def get_nc(att_mode=ATT_MODE, mm3_mode=MM3_MODE):
    key = (att_mode, mm3_mode)
    if key not in _NC_CACHE:
        _NC_CACHE[key] = build(att_mode, mm3_mode)
    return _NC_CACHE[key]


def make_in_maps(queries, keys, values, W, b):
    queries = np.ascontiguousarray(np.asarray(queries, dtype=np.float32))
    keys = np.ascontiguousarray(np.asarray(keys, dtype=np.float32))
    values = np.ascontiguousarray(np.asarray(values, dtype=np.float32))
    W = np.ascontiguousarray(np.asarray(W, dtype=np.float32))
    b = np.ascontiguousarray(np.asarray(b, dtype=np.float32))
    in_maps = []
    for i in range(N_CORES):
        s = slice(i * BPC, (i + 1) * BPC)
        in_maps.append(
            {
                "queries": queries[s],
                "keys": keys[s],
                "values": values[s],
                "W": W,
                "b": b,
            }
        )
    return in_maps


def kernel(queries, keys, values, W, b, **run_kwargs):
    nc = get_nc()
    in_maps = make_in_maps(queries, keys, values, W, b)
    res = run_bass_kernel_spmd(nc, in_maps, core_ids=list(range(N_CORES)), **run_kwargs)
    out = np.concatenate([res.results[i]["out"] for i in range(N_CORES)], axis=0)
    return out.reshape(B, L, C)


# revision 27
# speedup vs baseline: 1.1549x; 1.1549x over previous
"""Trainium2 Bass kernel for nn_AttentionLayer (B=128,H=16,L=64,E=128, C=2048).

out[b,l,:] = (softmax(0.1 * q_bh @ k_bh^T) @ v_bh  for h) . W^T + bias

Strategy: pure data-parallel over batch across 8 NeuronCores (16 batches
per core, no collectives).  Per core, in 8 two-batch blocks:
  - q/k/v DMA'd token-major (f32, sync queue), cast to fp16 on DVE.
    fp16 (not bf16): same PE speed, ~8x lower quantization error.
  - attention per (batch, head-pair) group in "scores^T" orientation:
    per-block q/k PE-transposes into one [e, hl] buffer, then one full
    128x128 k^T q matmul per group (diagonal 64x64 blocks are the two
    heads; off-diagonal garbage is never read),
  - softmax without max-subtraction (|0.1*s| < ~8 so exp cannot
    overflow): exp writes the diagonal blocks of a pre-zeroed ring slot
    so U = exp @ [v|1] can contract all 128 partitions in one matmul;
    the ones-column yields the rowsum; normalization is a per-partition
    tensor_scalar multiply in token-major form; V^T via a 16-bit PE
    transpose (the old f32 transpose was 2x the PE cycles),
  - output projection  out = V @ W^T + b  as K=2048 accumulated matmuls
    emitted n-slice-major (psum [128 tok, 512 n]); proj matmuls
    interleave between the NEXT blocks' attention groups (deque of
    generators) to keep the PE dense,
  - W^T prep as an engine-isolated pipeline that chases the W stream:
    W chunks on the gpsimd SWDGE queue (f32, concurrent with the sync
    queue's q/k/v loads), f32->fp16 cast on the Pool engine, PE
    transposes + psum->sbuf copies emitted as generators interleaved
    into blocks 0-2's attention groups,
  - bias add on DVE (Pool cannot read PSUM), store on the gpsimd queue.
"""

from collections import deque

import numpy as np

import concourse.bass as bass
import concourse.mybir as mybir
import concourse.tile as tile
from concourse import bacc
from concourse.bass_utils import run_bass_kernel_spmd
from concourse.masks import make_identity

N_CORES = 8
B, H, L, E = 128, 16, 64, 128
C = H * E                 # 2048
BPC = B // N_CORES        # 16 batches per core
NBLK = BPC // 2           # 8 two-batch blocks per core
G = H // 2                # 8 head-pair groups per batch
SCALE = 0.1
F32 = mybir.dt.float32
DT16 = mybir.dt.float16

# kept for test.py compatibility
ATT_MODE = "fp16"
MM3_MODE = "fp16"

EXPR = 8                  # exp ring slots


def emit(ctx, nc, tc, q_d, k_d, v_d, w_d, b_d, o_d):
    # DRAM views: [p, b, g, e] where token row (h*64+l) = g*128 + p
    qv = q_d.rearrange("b h l e -> b (h l) e").rearrange("b (g p) e -> p b g e", p=128)
    kv = k_d.rearrange("b h l e -> b (h l) e").rearrange("b (g p) e -> p b g e", p=128)
    vv = v_d.rearrange("b h l e -> b (h l) e").rearrange("b (g p) e -> p b g e", p=128)

    const = ctx.enter_context(tc.tile_pool(name="const", bufs=1))
    wst = ctx.enter_context(tc.tile_pool(name="wst", bufs=2))
    qkvf = ctx.enter_context(tc.tile_pool(name="qkvf", bufs=5))
    qkvb = ctx.enter_context(tc.tile_pool(name="qkvb", bufs=2))
    qktp = ctx.enter_context(tc.tile_pool(name="qktp", bufs=2))
    vtp = ctx.enter_context(tc.tile_pool(name="vtp", bufs=3))
    asml = ctx.enter_context(tc.tile_pool(name="asml", bufs=6))
    outp = ctx.enter_context(tc.tile_pool(name="outp", bufs=3))

    pat = ctx.enter_context(tc.tile_pool(name="pat", bufs=6, space="PSUM"))
    pps = pat
    pmm3 = ctx.enter_context(tc.tile_pool(name="pmm3", bufs=2, space="PSUM"))

    # ---- constants ----
    identity16 = const.tile([128, 128], DT16, tag="id16")
    make_identity(nc, identity16)

    # ring of pre-zeroed exp tiles: only the two diagonal 64x64 blocks are
    # ever (re)written, so the off-diagonal blocks stay zero and MM2 can
    # contract over the full 128 partitions without mixing the two heads
    exp_ring = const.tile([128, EXPR, 128], DT16, tag="expr")
    nc.vector.memset(exp_ring, 0.0)

    bias_bc = const.tile([128, C], F32, tag="bias")
    b_bcast = bass.AP(tensor=b_d.tensor, offset=b_d.offset, ap=[[0, 128]] + list(b_d.ap))
    nc.gpsimd.dma_start(out=bias_bc, in_=b_bcast)

    # W^T in fp16: wt_sb[p, kk, n] = W[n, kk*128+p]
    wt_sb = const.tile([128, H, C], DT16, tag="wt")

    # ---- block loads: DMA f32 (sync queue), cast to fp16 (DVE), PE
    # transposes of q/k into one [e, hl] buffer ----
    def load_block(m):
        qf = qkvf.tile([128, 2, G, 128], F32, tag="qkvf")
        kf = qkvf.tile([128, 2, G, 128], F32, tag="qkvf")
        vf = qkvf.tile([128, 2, G, 128], F32, tag="qkvf")
        qb = qkvb.tile([128, 2, G, 128], DT16, tag="qb")
        kb = qkvb.tile([128, 2, G, 128], DT16, tag="kb")
        vb = qkvb.tile([128, 2, G, 132], DT16, tag="vb")
        # per-batch-half loads so the first casts/transposes start earlier;
        # q/k casts ride the (otherwise idle) Pool engine, v stays on DVE
        for bb in range(2):
            nc.sync.dma_start(out=qf[:, bb], in_=qv[:, 2 * m + bb, :, :])
            nc.sync.dma_start(out=kf[:, bb], in_=kv[:, 2 * m + bb, :, :])
            nc.sync.dma_start(out=vf[:, bb], in_=vv[:, 2 * m + bb, :, :])
            nc.gpsimd.tensor_copy(qb[:, bb, :, :], qf[:, bb, :, :])
            nc.gpsimd.tensor_copy(kb[:, bb, :, :], kf[:, bb, :, :])
            nc.vector.tensor_copy(vb[:, bb, :, 0:128], vf[:, bb, :, :])
        nc.vector.memset(vb[:, :, :, 128:129], 1.0)
        return qb, kb, vb

    # ---- W prep: engine-isolated stream.  gpsimd queue: W f32 loads.
    # Pool: f32->fp16 cast.  PE: transposes (as generators interleaved into
    # blocks 0-2's attention).  psum->sbuf copies alternate scalar/DVE. ----
    def wprep_load(nt):
        wn_f = wst.tile([128, C], F32, tag="wnf", name=f"wnf{nt}")
        nc.gpsimd.dma_start(out=wn_f, in_=w_d[nt * 128 : (nt + 1) * 128, :])
        wn_c = wst.tile([128, C], DT16, tag="wnc", name=f"wnc{nt}")
        for hh in range(2):
            nc.scalar.copy(
                wn_c[:, hh * 1024 : (hh + 1) * 1024],
                wn_f[:, hh * 1024 : (hh + 1) * 1024],
            )
        return wn_c

    def wprep_emitter(nt, wn_c):
        for kk in range(0, H, 2):
            tp = pps.tile([128, 256], DT16, tag="at", name="wtp")
            nc.tensor.transpose(
                tp[:, 0:128], wn_c[:, kk * 128 : (kk + 1) * 128], identity16
            )
            yield
            nc.tensor.transpose(
                tp[:, 128:256], wn_c[:, (kk + 1) * 128 : (kk + 2) * 128], identity16
            )
            yield
            if (kk // 2) % 2 == 0:
                nc.scalar.copy(
                    wt_sb[:, kk : kk + 2, nt * 128 : (nt + 1) * 128],
                    tp.rearrange("p (a b) -> p a b", a=2),
                )
            else:
                nc.vector.tensor_copy(
                    wt_sb[:, kk : kk + 2, nt * 128 : (nt + 1) * 128],
                    tp.rearrange("p (a b) -> p a b", a=2),
                )
            yield
        # chain: queue the next W chunk's load + generator (ring depth 2)
        if nt + 2 < H:
            wpend.append(wprep_emitter(nt + 2, wprep_load(nt + 2)))

    # ---- output projection, n-slice-major; generator interleaves with the
    # NEXT blocks' attention groups ----
    def proj_emitter(m, vt):
        for nn in range(4):
            ps = pmm3.tile([128, 512], F32, tag="mm3", name=f"ps{nn}")
            for kk in range(H):
                nc.tensor.matmul(
                    ps,
                    vt[:, kk, :],
                    wt_sb[:, kk, nn * 512 : (nn + 1) * 512],
                    start=(kk == 0), stop=(kk == 15),
                )
                yield
            oth = outp.tile([128, 512], F32, tag="ot", name="oth")
            nc.vector.tensor_add(oth, ps, bias_bc[:, nn * 512 : (nn + 1) * 512])
            nc.gpsimd.dma_start(
                out=o_d[m * 128 : (m + 1) * 128, nn * 512 : (nn + 1) * 512],
                in_=oth,
            )
            yield

    def drain_steps(dq, k):
        while k > 0 and dq:
            try:
                next(dq[0])
                k -= 1
            except StopIteration:
                dq.popleft()

    def drain_all(dq):
        while dq:
            drain_steps(dq, 1 << 30)

    pending = deque()   # proj generators
    wpend = deque()     # wprep generators

    with nc.named_scope("load0"):
        blk_tiles = load_block(0)
    for nt in range(2):
        wpend.append(wprep_emitter(nt, wprep_load(nt)))

    # blocks 0-1 interleave wprep generator steps into attention groups;
    # each nt generator also queues the nt+3 W load when it finishes.
    # All wprep must be EMITTED before proj(0)'s later slices drain (Tile
    # only sees writers emitted before a reader), hence drain_all below.
    WSTEPS = {0: 12, 1: 16}
    PCAD = {0: 2, 1: 3, 2: 6, 3: 6}

    # ---- per-block pipeline ----
    for m in range(NBLK):
        qb, kb, vb = blk_tiles
        vt = vtp.tile([128, H, 128], DT16, tag="vt")
        with nc.named_scope(f"attn{m}"):
            # batch-transpose this block's q and k up front
            qkt = qktp.tile([128, 2, G, 256], DT16, tag="qkt")
            for bb in range(2):
                for g in range(G):
                    trp = pps.tile([128, 256], DT16, tag="at", name="trp")
                    nc.tensor.transpose(trp[:, 0:128], qb[:, bb, g, :], identity16)
                    nc.tensor.transpose(trp[:, 128:256], kb[:, bb, g, :], identity16)
                    nc.vector.tensor_copy(qkt[:, bb, g, :], trp)
            for bb in range(2):
                for g in range(G):
                    drain_steps(wpend, WSTEPS.get(m, 0))
                    drain_steps(pending, PCAD.get(m, 5))
                    qT2 = qkt[:, bb, g, 0:128]
                    kT2 = qkt[:, bb, g, 128:256]

                    # One psum bank holds this group's scores^T (cols 0:128),
                    # U' = exp@[v|1] (cols 128:257), V^T fp16 (bitcast region)
                    at = pat.tile([128, 392], F32, tag="at")
                    scT = at[:, 0:128]
                    nc.tensor.matmul(scT, kT2, qT2, start=True, stop=True)

                    # exp(scale * scores^T) into a pre-zeroed ring slot
                    expT = exp_ring[:, (bb * G + g) % EXPR, :]
                    for lo, hi in ((0, 64), (64, 128)):
                        nc.scalar.activation(
                            expT[lo:hi, lo:hi], scT[lo:hi, lo:hi],
                            mybir.ActivationFunctionType.Exp, scale=SCALE,
                        )

                    # U = exp @ [v | 1]  -> token-major U plus rowsum column
                    U2p = at[:, 128:257]
                    nc.tensor.matmul(
                        U2p, expT, vb[:, bb, g, 0:129], start=True, stop=True
                    )

                    # normalize token-major (per-partition scalar), fp16 out
                    r2 = asml.tile([128, 1], F32, tag="r2")
                    nc.vector.reciprocal(r2, U2p[:, 128:129])
                    V2 = asml.tile([128, 128], DT16, tag="V2")
                    nc.vector.tensor_scalar_mul(V2, U2p[:, 0:128], r2)

                    # 16-bit PE transpose of V into c-major layout
                    VT2p = at[:, 260:324].bitcast(DT16)
                    nc.tensor.transpose(VT2p, V2, identity16)
                    tok = bb * 64
                    nc.vector.tensor_copy(
                        vt[:, 2 * g : 2 * g + 2, tok : tok + 64],
                        VT2p.rearrange("p (a b) -> p a b", a=2),
                    )

        # drain any wprep leftovers before proj(0)'s later slices emit
        if m == 1:
            drain_all(wpend)
        # prefetch next block
        if m + 1 < NBLK:
            with nc.named_scope(f"load{m + 1}"):
                blk_tiles = load_block(m + 1)
        pending.append(proj_emitter(m, vt))
        if m == NBLK - 1:
            drain_all(pending)


def build(att_mode=ATT_MODE, mm3_mode=MM3_MODE):
    import contextlib

    nc = bacc.Bacc("TRN2", target_bir_lowering=False, debug=False)
    q_d = nc.dram_tensor("queries", [BPC, H, L, E], F32, kind="ExternalInput").ap()
    k_d = nc.dram_tensor("keys", [BPC, H, L, E], F32, kind="ExternalInput").ap()
    v_d = nc.dram_tensor("values", [BPC, H, L, E], F32, kind="ExternalInput").ap()
    w_d = nc.dram_tensor("W", [C, C], F32, kind="ExternalInput").ap()
    b_d = nc.dram_tensor("b", [C], F32, kind="ExternalInput").ap()
    o_d = nc.dram_tensor("out", [BPC * L, C], F32, kind="ExternalOutput").ap()

    with tile.TileContext(nc) as tc:
        with contextlib.ExitStack() as ctx:
            emit(ctx, nc, tc, q_d, k_d, v_d, w_d, b_d, o_d)
    nc.compile()
    return nc


_NC_CACHE = {}


def get_nc(att_mode=ATT_MODE, mm3_mode=MM3_MODE):
    key = (att_mode, mm3_mode)
    if key not in _NC_CACHE:
        _NC_CACHE[key] = build(att_mode, mm3_mode)
    return _NC_CACHE[key]


def make_in_maps(queries, keys, values, W, b):
    queries = np.ascontiguousarray(np.asarray(queries, dtype=np.float32))
    keys = np.ascontiguousarray(np.asarray(keys, dtype=np.float32))
    values = np.ascontiguousarray(np.asarray(values, dtype=np.float32))
    W = np.ascontiguousarray(np.asarray(W, dtype=np.float32))
    b = np.ascontiguousarray(np.asarray(b, dtype=np.float32))
    in_maps = []
    for i in range(N_CORES):
        s = slice(i * BPC, (i + 1) * BPC)
        in_maps.append(
            {
                "queries": queries[s],
                "keys": keys[s],
                "values": values[s],
                "W": W,
                "b": b,
            }
        )
    return in_maps


def kernel(queries, keys, values, W, b, **run_kwargs):
    nc = get_nc()
    in_maps = make_in_maps(queries, keys, values, W, b)
    res = run_bass_kernel_spmd(nc, in_maps, core_ids=list(range(N_CORES)), **run_kwargs)
    out = np.concatenate([res.results[i]["out"] for i in range(N_CORES)], axis=0)
    return out.reshape(B, L, C)
